# revision 1
# baseline (speedup 1.0000x reference)
"""Causal self-attention on 8 Trainium2 NeuronCores (Bass/Tile).

Problem: nn_CausalSelfAttention (B=4, T=2048, C=1024, H=16 heads, fp32).

Sharding: tensor-parallel over heads for QKV projection + attention
(2 heads per core), per-batch AllGather of attention outputs (fp16,
transposed layout), then tensor-parallel over output columns for the
final projection (each core computes a 128-column slice of x@W_proj).
The AllGather/projection of batch b runs concurrently with the
attention of batch b+1.

Layouts (feature dim on partitions everywhere):
  xT      [C, B*T]        input, replicated to all cores
  Q^T,K^T [CH, B*T]       CH = channels per core (2 heads x 64)
  V       [B*T, CH]       natural layout (matmul lhsT for P@V), stored
                          tiled with an extra ones-column per head so the
                          PV matmul also produces softmax denominators
  S^T     [kr, q] tiles   scores transposed: kr on partitions -> exp'd
                          tiles feed P@V directly as the moving operand
  attn^T  [CH, T] fp16    per-core, per-batch -> AllGather -> [C, T]
  y^T     [OC, B*T]       per-core 128-column slice of the final output

Softmax: unnormalized exp (scores are O(1), no max subtraction needed);
causal mask = DVE add of a -1e5 upper-triangular [128,128] constant onto
the single diagonal-crossing strip of each clipped tile; denominator from
the V ones-column; division via reciprocal_approx_fast + PE broadcast.
Matmuls run as float32r (single-pass fp32); proj runs fp16 inputs.
"""

import numpy as np
from contextlib import ExitStack

P = 128
NQ = 512  # q/moving-operand tile width
MASKVAL = -1.0e5


def build_attention_nc(B, T, C, H, n_cores):
    import concourse.bass as bass  # noqa: F401
    import concourse.bacc as bacc
    import concourse.tile as tile
    import concourse.mybir as mybir

    f32 = mybir.dt.float32
    f32r = mybir.dt.float32r
    fp16 = mybir.dt.float16
    Identity = mybir.ActivationFunctionType.Identity
    Exp = mybir.ActivationFunctionType.Exp

    hs = C // H              # head size
    hpc = H // n_cores       # heads per core
    CH = hpc * hs            # qkv channels per core
    OC = C // n_cores        # output columns per core
    NT = B * T               # tokens
    KT_E = C // P            # contraction tiles over embedding dim
    NROW = NT // NQ          # token row-tiles
    TQ = T // NQ             # q tiles per batch
    TK = T // P              # kr tiles per batch
    TKALL = NT // P          # kr tiles over all batches
    DPB = NQ // P            # kr-tiles crossing one q-tile's diagonal
    WV = hpc * (hs + 1)      # V storage width per kr-tile (with ones cols)

    assert T % NQ == 0 and C % P == 0 and NT % NQ == 0
    assert CH <= P and H % n_cores == 0
    assert hs * hpc == CH and hpc in (1, 2)
    scale = 1.0 / float(np.sqrt(hs))

    nc = bacc.Bacc("TRN2", target_bir_lowering=False, debug=False,
                   num_devices=n_cores)

    xT = nc.dram_tensor("xT", [C, NT], f32r, kind="ExternalInput")
    wqkv = nc.dram_tensor("wqkv", [C, 3 * CH], f32r, kind="ExternalInput")
    bqkv = nc.dram_tensor("bqkv", [CH, 3], f32, kind="ExternalInput")
    wp = nc.dram_tensor("wp", [C, OC], fp16, kind="ExternalInput")
    bp = nc.dram_tensor("bp", [OC, 1], f32, kind="ExternalInput")
    yT = nc.dram_tensor("yT", [OC, NT], f32, kind="ExternalOutput")

    # constants baked into the NEFF
    ident_np = np.eye(P, dtype=np.float32)
    # mask[p, c] = MASKVAL where kr-offset p > q-offset c (strictly lower)
    mask_np = np.ascontiguousarray(np.where(
        np.arange(P)[:, None] > np.arange(P)[None, :], MASKVAL, 0.0
    ).astype(np.float32))
    ident_dram = nc.inline_tensor(ident_np.astype(np.float16),
                                  name="ident_const")
    mask_dram = nc.inline_tensor(
        np.where(mask_np < 0, np.float16(-60000.0),
                 np.float16(0.0)).astype(np.float16), name="mask_const")
    ones_dram = nc.inline_tensor(np.ones((P, hs), dtype=np.float32),
                                 name="ones_const")
    vones_dram = nc.inline_tensor(np.ones((P, TKALL * hpc), dtype=np.float16),
                                  name="vones_const")

    with tile.TileContext(nc) as tc, ExitStack() as ctx:
        const = ctx.enter_context(tc.tile_pool(name="const", bufs=1))
        big = ctx.enter_context(tc.tile_pool(name="big", bufs=1))
        xin = ctx.enter_context(tc.tile_pool(name="xin", bufs=2))
        evac = ctx.enter_context(tc.tile_pool(name="evac", bufs=3))
        pexp = ctx.enter_context(tc.tile_pool(name="pexp", bufs=6))
        stp = ctx.enter_context(tc.tile_pool(name="stp", bufs=4, space="PSUM"))
        pvp = ctx.enter_context(tc.tile_pool(name="pvp", bufs=4, space="PSUM"))
        dram = ctx.enter_context(tc.tile_pool(name="dram", bufs=1, space="DRAM"))

        ident_t = const.tile([P, P], fp16)
        mask_sb = const.tile([P, P], fp16)
        ones_sb = const.tile([P, hs], f32r)
        bqkv_sb = const.tile([CH, 3], f32)
        bp_sb = const.tile([OC, 1], f32)
        w_sb = const.tile([P, KT_E * 3 * CH], f32r)
        wp_sb = const.tile([P, KT_E * OC], fp16)

        nc.sync.dma_start(ident_t[:], ident_dram[:])
        nc.sync.dma_start(mask_sb[:], mask_dram[:])
        nc.sync.dma_start(ones_sb[:], ones_dram[:].bitcast(f32r))
        nc.sync.dma_start(bqkv_sb[:], bqkv[:])
        nc.sync.dma_start(bp_sb[:], bp[:])
        nc.sync.dma_start(
            w_sb[:].rearrange("p (k m) -> p k m", k=KT_E),
            wqkv[:].rearrange("(k p) m -> p k m", p=P),
        )
        nc.sync.dma_start(
            wp_sb[:].rearrange("p (k m) -> p k m", k=KT_E),
            wp[:].rearrange("(k p) m -> p k m", p=P),
        )

        # per-head zero-padded Q^T blocks: full K=128 contraction keeps the
        # PE activity monitor warm (K=64 matmuls throttle the clock to 1/2)
        QTp = big.tile([P, hpc * NT], fp16)
        KTp = big.tile([P, NT], fp16)
        VT = big.tile([CH, NT], fp16)
        Vn = big.tile([P, TKALL * WV], fp16)
        def memset_rows(ap2d, a, b):
            # split [a, b) partition ranges on 32/64/96 boundaries so each
            # memset satisfies the base-partition span rules
            cuts = [c for c in (32, 64, 96) if a < c < b]
            for lo, hi in zip([a] + cuts, cuts + [b]):
                nc.any.memset(ap2d[lo:hi, :], 0.0)

        if hpc == 2:
            memset_rows(QTp[:, NT:2 * NT], 0, hs)
            memset_rows(QTp[:, 0:NT], hs, min(CH, P) if CH < P else P)
            if CH < P:
                memset_rows(QTp[:, 0:NT], CH, P)
                memset_rows(QTp[:, NT:2 * NT], CH, P)
        elif CH < P:
            memset_rows(QTp[:, :], CH, P)
        if CH < P:
            memset_rows(KTp[:, :], CH, P)

        # ones columns of V (softmax denominator trick)
        ones_view = Vn[:].rearrange("p (v h d) -> p v h d", h=hpc, d=hs + 1)[
            :, :, :, hs:hs + 1
        ]
        nc.sync.dma_start(
            ones_view,
            vones_dram[:].rearrange("p (v h d) -> p v h d", h=hpc, d=1),
        )

        # ---- Phase A: QKV projection (outputs transposed) ----
        for n in range(NROW):
            ns = n * NQ
            xt = xin.tile([P, KT_E * NQ], f32r, tag="xcol")
            nc.sync.dma_start(
                xt[:].rearrange("p (k q) -> p k q", k=KT_E),
                xT[:, ns:ns + NQ].rearrange("(k p) q -> p k q", p=P),
            )
            for m in range(3):
                ps = stp.tile([P, NQ], f32, tag="st")
                for k in range(KT_E):
                    nc.tensor.matmul(
                        ps[0:CH, :],
                        lhsT=w_sb[:, k * 3 * CH + m * CH:k * 3 * CH + (m + 1) * CH],
                        rhs=xt[:, k * NQ:(k + 1) * NQ],
                        start=(k == 0),
                        stop=(k == KT_E - 1),
                    )
                if m == 0:
                    for hh in range(hpc):
                        nc.scalar.activation(
                            QTp[hh * hs:(hh + 1) * hs, hh * NT + ns:
                                hh * NT + ns + NQ],
                            ps[hh * hs:(hh + 1) * hs, :],
                            Identity, bias=bqkv_sb[hh * hs:(hh + 1) * hs,
                                                   0:1])
                elif m == 1:
                    nc.scalar.activation(KTp[0:CH, ns:ns + NQ], ps[0:CH, :],
                                         Identity, bias=bqkv_sb[:, 1:2])
                else:
                    nc.scalar.activation(VT[:, ns:ns + NQ], ps[0:CH, :],
                                         Identity, bias=bqkv_sb[:, 2:3])

        # V transposes, clustered into one burst (PE transpose-mode does not
        # count as PE-busy for the clock gate; one dip beats sixteen)
        for n in range(NROW):
            tp = stp.tile([P, DPB * CH], fp16, tag="st", name="tp")
            for j in range(DPB):
                nc.tensor.transpose(
                    tp[:, j * CH:(j + 1) * CH],
                    VT[:, (n * DPB + j) * P:(n * DPB + j + 1) * P],
                    ident_t[0:CH, 0:CH],
                )
            vi0 = n * DPB
            dst = Vn[:, vi0 * WV:(vi0 + DPB) * WV].rearrange(
                "p (v h d) -> p v h d", h=hpc, d=hs + 1
            )[:, :, :, 0:hs]
            nc.vector.tensor_copy(dst, tp[:].rearrange(
                "p (v h d) -> p v h d", h=hpc, d=hs))

        # ---- Phase B + C interleaved per batch ----
        cc_ins = [dram.tile([CH, T], fp16, name=f"ccin{b}") for b in range(B)]
        cc_outs = [dram.tile([n_cores * CH, T], fp16, addr_space="Shared",
                             name=f"ccout{b}") for b in range(B)]

        def attention(b):
            # normalization of q-tile qt-1 is deferred into qt's kt-loop so
            # the in-order PE never stalls behind the 3.3us DVE reciprocal
            # (which would re-throttle the HAM clock gate every tile).
            prev = None

            def issue_recips(state):
                qt0, pvs0, recs0 = state
                for hh in range(hpc):
                    with nc.allow_low_precision(reason="recip for bcast"):
                        nc.vector.reciprocal(recs0[hh][hs:hs + 1, :],
                                             pvs0[hh][hs:hs + 1, :])

            def issue_norm_rest(state):
                qt0, pvs0, recs0 = state
                for hh in range(hpc):
                    bc = stp.tile([P, NQ], f32, tag="st")
                    nc.tensor.matmul(
                        bc[0:hs, :],
                        lhsT=ones_sb[hs:hs + 1, :],
                        rhs=recs0[hh][hs:hs + 1, :],
                        start=True,
                        stop=True,
                    )
                    bcs = evac.tile([hs, NQ], f32, tag="bcs")
                    nc.vector.tensor_copy(bcs[:], bc[0:hs, :])
                    ao = evac.tile([hs, NQ], fp16, tag="ao")
                    nc.vector.tensor_mul(ao[:], pvs0[hh][0:hs, :], bcs[:])
                    nc.sync.dma_start(
                        cc_ins[b][hh * hs:(hh + 1) * hs,
                                  qt0 * NQ:(qt0 + 1) * NQ],
                        ao[:],
                    )

            for qt in range(TQ):
                qs = b * T + qt * NQ
                nkt = DPB * qt + DPB
                pvs = [pvp.tile([P, NQ], f32, tag="pv", name=f"pv{_h}")
                       for _h in range(hpc)]
                pes = {}

                def issue_st(kt, qt=qt, qs=qs, pes=pes):
                    ks = b * T + kt * P
                    diag = kt >= DPB * qt
                    j = kt - DPB * qt
                    c0 = j * P if diag else 0
                    for hh in range(hpc):
                        st = stp.tile([P, NQ], f32, tag="st", name=f"st{hh}")
                        nc.tensor.matmul(
                            st[:, c0:NQ],
                            lhsT=KTp[:, ks:ks + P],
                            rhs=QTp[:, hh * NT + qs + c0:hh * NT + qs + NQ],
                            start=True,
                            stop=not diag,
                        )
                        if diag:
                            nc.tensor.matmul(
                                st[:, c0:c0 + P],
                                lhsT=ident_t[:],
                                rhs=mask_sb[:],
                                start=False,
                                stop=True,
                            )
                        pe_t = pexp.tile([P, NQ], fp16, tag="pe",
                                         name=f"pe{hh}")
                        nc.scalar.activation(pe_t[:, c0:NQ], st[:, c0:NQ],
                                             Exp, scale=scale)
                        pes[(kt, hh)] = (pe_t, c0)

                def issue_pv(kt, nkt=nkt, pvs=pvs, pes=pes):
                    vi = b * TK + kt
                    for hh in range(hpc):
                        pe_t, c0 = pes.pop((kt, hh))
                        nc.tensor.matmul(
                            pvs[hh][0:hs + 1, c0:NQ],
                            lhsT=Vn[:, vi * WV + hh * (hs + 1):
                                    vi * WV + (hh + 1) * (hs + 1)],
                            rhs=pe_t[:, c0:NQ],
                            start=(kt == 0),
                            stop=(kt == nkt - 1),
                        )

                for kt in range(nkt):
                    issue_st(kt)
                    if kt == 3 and prev is not None:
                        issue_norm_rest(prev)
                    if kt >= 2:
                        issue_pv(kt - 2)
                for kt in range(max(0, nkt - 2), nkt):
                    issue_pv(kt)

                recs = [evac.tile([P, NQ], f32r, tag="rec", name=f"rec{_h}",
                                  bufs=2 * hpc)
                        for _h in range(hpc)]
                prev = (qt, pvs, recs)
                issue_recips(prev)

            issue_norm_rest(prev)

        def proj(b):
            for n in range(T // NQ):
                ns = n * NQ
                rt = xin.tile([P, KT_E * NQ], fp16, tag="xcol")
                nc.sync.dma_start(
                    rt[:].rearrange("p (k q) -> p k q", k=KT_E),
                    cc_outs[b][:, ns:ns + NQ].rearrange("(k p) q -> p k q",
                                                        p=P),
                )
                ps = stp.tile([P, NQ], f32, tag="st")
                for k in range(KT_E):
                    nc.tensor.matmul(
                        ps[0:OC, :],
                        lhsT=wp_sb[:, k * OC:(k + 1) * OC],
                        rhs=rt[:, k * NQ:(k + 1) * NQ],
                        start=(k == 0),
                        stop=(k == KT_E - 1),
                    )
                yo = evac.tile([OC, NQ], f32, tag="yo")
                nc.scalar.activation(yo[:], ps[0:OC, :], Identity,
                                     bias=bp_sb[:, 0:1])
                nc.sync.dma_start(yT[:, b * T + ns:b * T + ns + NQ], yo[:])

        for b in range(B):
            attention(b)
            nc.gpsimd.collective_compute(
                "AllGather",
                mybir.AluOpType.bypass,
                replica_groups=[list(range(n_cores))],
                ins=[cc_ins[b][:].opt()],
                outs=[cc_outs[b][:].opt()],
            )
            if b >= 1:
                proj(b - 1)
        proj(B - 1)

    nc.compile()
    return nc


def shard_inputs(x, W_qkv, b_qkv, W_proj, b_proj, H, n_cores):
    B, T, C = x.shape
    hs = C // H
    hpc = H // n_cores
    CH = hpc * hs
    OC = C // n_cores
    x2 = np.asarray(x, dtype=np.float32).reshape(B * T, C)
    xT = np.ascontiguousarray(x2.T)
    W_qkv = np.asarray(W_qkv, dtype=np.float32)
    b_qkv = np.asarray(b_qkv, dtype=np.float32)
    W_proj = np.asarray(W_proj, dtype=np.float32)
    b_proj = np.asarray(b_proj, dtype=np.float32)
    in_maps = []
    for i in range(n_cores):
        sl = slice(i * CH, (i + 1) * CH)
        wqkv_i = np.ascontiguousarray(np.concatenate(
            [W_qkv[:, sl], W_qkv[:, C:][:, sl], W_qkv[:, 2 * C:][:, sl]], axis=1))
        bqkv_i = np.ascontiguousarray(np.stack(
            [b_qkv[sl], b_qkv[C:][sl], b_qkv[2 * C:][sl]], axis=1))
        wp_i = np.ascontiguousarray(
            W_proj[:, i * OC:(i + 1) * OC].astype(np.float16))
        bp_i = np.ascontiguousarray(b_proj[i * OC:(i + 1) * OC].reshape(OC, 1))
        in_maps.append({"xT": xT, "wqkv": wqkv_i, "bqkv": bqkv_i,
                        "wp": wp_i, "bp": bp_i})
    return in_maps


def gather_output(results, B, T, C, n_cores):
    yT = np.concatenate([results[i]["yT"] for i in range(n_cores)], axis=0)
    return np.ascontiguousarray(yT.T).reshape(B, T, C).astype(np.float32)


_NC_CACHE = {}


def _get_nc(B, T, C, H, n_cores):
    key = (B, T, C, H, n_cores)
    if key not in _NC_CACHE:
        _NC_CACHE[key] = build_attention_nc(B, T, C, H, n_cores)
    return _NC_CACHE[key]


def kernel(x, W_qkv, b_qkv, W_proj, b_proj):
    from concourse import bass_utils

    B, T, C = 4, 2048, 1024
    H, n_cores = 16, 8
    assert x.shape == (B, T, C)
    nc = _get_nc(B, T, C, H, n_cores)
    in_maps = shard_inputs(x, W_qkv, b_qkv, W_proj, b_proj, H, n_cores)
    res = bass_utils.run_bass_kernel_spmd(
        nc, in_maps, core_ids=list(range(n_cores))
    )
    return gather_output(res.results, B, T, C, n_cores)



# revision 8
# speedup vs baseline: 1.0696x; 1.0696x over previous
"""Causal self-attention on 8 Trainium2 NeuronCores (Bass/Tile).

Problem: nn_CausalSelfAttention (B=4, T=2048, C=1024, H=16 heads, fp32).

Sharding: tensor-parallel over heads for QKV projection + attention
(2 heads per core), per-batch AllGather of attention outputs (fp16,
transposed layout), then tensor-parallel over output columns for the
final projection (each core computes a 128-column slice of x@W_proj).
The AllGather/projection of batch b runs concurrently with the
attention of batch b+1.

Layouts (feature dim on partitions everywhere):
  xT      [C, B*T]        input (fp16), replicated to all cores
  Q^T,K^T [CH, B*T]       CH = channels per core (2 heads x 64), the two
                          heads stacked on partitions 0:64 / 64:128
  V       [B*T, CH]       natural layout (matmul lhsT for P@V), stored
                          tiled with an extra ones-column per head so the
                          PV matmul also produces softmax denominators;
                          filled via DMA-XBAR transposes (off the PE)
  S^T     [kr, q] tiles   scores transposed: kr on partitions -> exp'd
                          tiles feed P@V directly as the moving operand.
                          The two heads' score matmuls are K=64 row-tiled
                          pairs (tile_position (0,0)/(64,0)) that execute
                          concurrently on the PE.
  attn^T  [CH, T] fp16    per-core, per-batch -> AllGather -> [C, T]
  y^T     [OC, B*T]       per-core 128-column slice of the final output

Softmax: unnormalized exp (scores are O(1), no max subtraction needed);
causal mask = PE add of a -60000 upper-triangular [128,128] constant onto
the single diagonal-crossing strip of each clipped tile; denominator from
the V ones-column; division via reciprocal_approx_fast + PE broadcast.
All matmuls run fp16 inputs with fp32 PSUM accumulation.
"""

import numpy as np
from contextlib import ExitStack

P = 128
NQ = 512  # q/moving-operand tile width
MASKVAL = -60000.0


def build_attention_nc(B, T, C, H, n_cores):
    import concourse.bass as bass  # noqa: F401
    import concourse.bacc as bacc
    import concourse.tile as tile
    import concourse.mybir as mybir

    f32 = mybir.dt.float32
    f32r = mybir.dt.float32r
    fp16 = mybir.dt.float16
    Identity = mybir.ActivationFunctionType.Identity
    Exp = mybir.ActivationFunctionType.Exp

    hs = C // H              # head size
    hpc = H // n_cores       # heads per core
    CH = hpc * hs            # qkv channels per core
    OC = C // n_cores        # output columns per core
    NT = B * T               # tokens
    KT_E = C // P            # contraction tiles over embedding dim
    NROW = NT // NQ          # token row-tiles
    TQ = T // NQ             # q tiles per batch
    TK = T // P              # kr tiles per batch
    TKALL = NT // P          # kr tiles over all batches
    DPB = NQ // P            # kr-tiles crossing one q-tile's diagonal
    WV = hpc * (hs + 1)      # V storage width per kr-tile (with ones cols)

    assert T % NQ == 0 and C % P == 0 and NT % NQ == 0
    assert CH == P and H % n_cores == 0 and hpc == 2 and hs == 64
    scale = 1.0 / float(np.sqrt(hs))

    nc = bacc.Bacc("TRN2", target_bir_lowering=False, debug=False,
                   num_devices=n_cores)

    xT = nc.dram_tensor("xT", [C, NT], fp16, kind="ExternalInput")
    wqkv = nc.dram_tensor("wqkv", [C, 3 * CH], fp16, kind="ExternalInput")
    bqkv = nc.dram_tensor("bqkv", [CH, 3], f32, kind="ExternalInput")
    wp = nc.dram_tensor("wp", [C, OC], fp16, kind="ExternalInput")
    bp = nc.dram_tensor("bp", [OC, 1], f32, kind="ExternalInput")
    yT = nc.dram_tensor("yT", [OC, NT], f32, kind="ExternalOutput")

    # constants baked into the NEFF
    ident_np = np.eye(P, dtype=np.float16)
    # mask[p, c] = MASKVAL where kr-offset p > q-offset c (strictly lower)
    mask_np = np.where(
        np.arange(P)[:, None] > np.arange(P)[None, :],
        np.float16(MASKVAL), np.float16(0.0)).astype(np.float16)
    ident_dram = nc.inline_tensor(ident_np, name="ident_const")
    mask_dram = nc.inline_tensor(mask_np, name="mask_const")
    ones_dram = nc.inline_tensor(np.ones((P, hs), dtype=np.float16),
                                 name="ones_const")
    vones_dram = nc.inline_tensor(np.ones((P, TKALL * hpc), dtype=np.float16),
                                  name="vones_const")

    with tile.TileContext(nc) as tc, ExitStack() as ctx:
        const = ctx.enter_context(tc.tile_pool(name="const", bufs=1))
        big = ctx.enter_context(tc.tile_pool(name="big", bufs=1))
        xin = ctx.enter_context(tc.tile_pool(name="xin", bufs=2))
        evac = ctx.enter_context(tc.tile_pool(name="evac", bufs=3))
        pexp = ctx.enter_context(tc.tile_pool(name="pexp", bufs=6))
        stp = ctx.enter_context(tc.tile_pool(name="stp", bufs=4, space="PSUM"))
        pvp = ctx.enter_context(tc.tile_pool(name="pvp", bufs=4, space="PSUM"))
        dram = ctx.enter_context(tc.tile_pool(name="dram", bufs=1, space="DRAM"))

        ident_t = const.tile([P, P], fp16)
        mask_sb = const.tile([P, P], fp16)
        ones_sb = const.tile([P, hs], fp16)
        bqkv_sb = const.tile([CH, 3], f32)
        bp_sb = const.tile([OC, 1], f32)
        w_sb = const.tile([P, KT_E * 3 * CH], fp16)
        wp_sb = const.tile([P, KT_E * OC], fp16)

        nc.sync.dma_start(ident_t[:], ident_dram[:])
        nc.sync.dma_start(mask_sb[:], mask_dram[:])
        nc.sync.dma_start(ones_sb[:], ones_dram[:])
        nc.sync.dma_start(bqkv_sb[:], bqkv[:])
        nc.sync.dma_start(bp_sb[:], bp[:])
        nc.sync.dma_start(
            w_sb[:].rearrange("p (k m) -> p k m", k=KT_E),
            wqkv[:].rearrange("(k p) m -> p k m", p=P),
        )
        nc.sync.dma_start(
            wp_sb[:].rearrange("p (k m) -> p k m", k=KT_E),
            wp[:].rearrange("(k p) m -> p k m", p=P),
        )

        QT = big.tile([P, NT], fp16)
        KTp = big.tile([P, NT], fp16)
        VT = big.tile([CH, NT], fp16)
        Vn = big.tile([P, TKALL * WV], fp16)

        # ones columns of V (softmax denominator trick)
        ones_view = Vn[:].rearrange("p (v h d) -> p v h d", h=hpc, d=hs + 1)[
            :, :, :, hs:hs + 1
        ]
        nc.sync.dma_start(
            ones_view,
            vones_dram[:].rearrange("p (v h d) -> p v h d", h=hpc, d=1),
        )

        # ---- Phase A: QKV projection (outputs transposed) ----
        for n in range(NROW):
            ns = n * NQ
            xt = xin.tile([P, KT_E * NQ], fp16, tag="xcol")
            nc.sync.dma_start(
                xt[:].rearrange("p (k q) -> p k q", k=KT_E),
                xT[:, ns:ns + NQ].rearrange("(k p) q -> p k q", p=P),
            )
            for m in range(3):
                ps = stp.tile([P, NQ], f32, tag="st")
                for k in range(KT_E):
                    nc.tensor.matmul(
                        ps[:],
                        lhsT=w_sb[:, k * 3 * CH + m * CH:k * 3 * CH + (m + 1) * CH],
                        rhs=xt[:, k * NQ:(k + 1) * NQ],
                        start=(k == 0),
                        stop=(k == KT_E - 1),
                    )
                dst = (QT, KTp, VT)[m]
                nc.scalar.activation(dst[:, ns:ns + NQ], ps[:],
                                     Identity, bias=bqkv_sb[:, m:m + 1])
            # V transposes for this row-tile on the PE; interleaved with the
            # QKV matmul stream the per-row dip is ~1.1us, under the HAM MID
            # window, so the clock gate stays open
            tp = stp.tile([P, DPB * CH], fp16, tag="st", name="tp")
            for j in range(DPB):
                nc.tensor.transpose(
                    tp[:, j * CH:(j + 1) * CH],
                    VT[:, (n * DPB + j) * P:(n * DPB + j + 1) * P],
                    ident_t[:],
                )
            vi0 = n * DPB
            dst = Vn[:, vi0 * WV:(vi0 + DPB) * WV].rearrange(
                "p (v h d) -> p v h d", h=hpc, d=hs + 1
            )[:, :, :, 0:hs]
            nc.vector.tensor_copy(dst, tp[:].rearrange(
                "p (v h d) -> p v h d", h=hpc, d=hs))

        # ---- Phase B + C interleaved per batch ----
        cc_ins = [dram.tile([CH, T], fp16, name=f"ccin{b}") for b in range(B)]
        cc_outs = [dram.tile([n_cores * CH, T], fp16, addr_space="Shared",
                             name=f"ccout{b}") for b in range(B)]

        def attention(b):
            # normalization of q-tile qt-1 is deferred into qt's kt-loop so
            # the in-order PE never stalls behind the DVE reciprocal
            prev = None

            def issue_dens(state):
                # evacuate the two denominator rows (fp32 PSUM -> fp16 SBUF)
                qt0, pvs0, dens0 = state
                for hh in range(hpc):
                    nc.vector.tensor_copy(dens0[hh][:],
                                          pvs0[hh][hs:hs + 1, :])

            def issue_norm_rest(state):
                qt0, pvs0, dens0 = state
                for hh in range(hpc):
                    bc = stp.tile([P, NQ], f32, tag="st")
                    nc.tensor.matmul(
                        bc[0:hs, :],
                        lhsT=ones_sb[0:1, :],
                        rhs=dens0[hh][:],
                        start=True,
                        stop=True,
                    )
                    recs = evac.tile([hs, NQ], f32, tag="rec")
                    nc.vector.reciprocal_approx_fast(recs[:], bc[0:hs, :])
                    ao = evac.tile([hs, NQ], fp16, tag="ao")
                    nc.vector.tensor_mul(ao[:], pvs0[hh][0:hs, :], recs[:])
                    nc.sync.dma_start(
                        cc_ins[b][hh * hs:(hh + 1) * hs,
                                  qt0 * NQ:(qt0 + 1) * NQ],
                        ao[:],
                    )

            for qt in range(TQ):
                qs = b * T + qt * NQ
                nkt = DPB * qt + DPB
                pvs = [pvp.tile([P, NQ], f32, tag="pv", name=f"pv{_h}")
                       for _h in range(hpc)]
                pes = {}

                def issue_st(kt, qt=qt, qs=qs, pes=pes):
                    ks = b * T + kt * P
                    diag = kt >= DPB * qt
                    j = kt - DPB * qt
                    c0 = j * P if diag else 0
                    sts = []
                    for hh in range(hpc):
                        st = stp.tile([P, NQ], f32, tag="st", name=f"st{hh}")
                        nc.tensor.matmul(
                            st[:, c0:NQ],
                            lhsT=KTp[hh * hs:(hh + 1) * hs, ks:ks + P],
                            rhs=QT[hh * hs:(hh + 1) * hs, qs + c0:qs + NQ],
                            start=True,
                            stop=not diag,
                            tile_position=(hh * hs, 0),
                        )
                        sts.append(st)
                    for hh in range(hpc):
                        st = sts[hh]
                        if diag:
                            nc.tensor.matmul(
                                st[:, c0:c0 + P],
                                lhsT=ident_t[:],
                                rhs=mask_sb[:],
                                start=False,
                                stop=True,
                            )
                        pe_t = pexp.tile([P, NQ], fp16, tag="pe",
                                         name=f"pe{hh}")
                        nc.scalar.activation(pe_t[:, c0:NQ], st[:, c0:NQ],
                                             Exp, scale=scale)
                        pes[(kt, hh)] = (pe_t, c0)

                def issue_pv(kt, nkt=nkt, pvs=pvs, pes=pes):
                    vi = b * TK + kt
                    for hh in range(hpc):
                        pe_t, c0 = pes.pop((kt, hh))
                        nc.tensor.matmul(
                            pvs[hh][0:hs + 1, c0:NQ],
                            lhsT=Vn[:, vi * WV + hh * (hs + 1):
                                    vi * WV + (hh + 1) * (hs + 1)],
                            rhs=pe_t[:, c0:NQ],
                            start=(kt == 0),
                            stop=(kt == nkt - 1),
                        )

                for kt in range(nkt):
                    issue_st(kt)
                    if kt == 3 and prev is not None:
                        issue_norm_rest(prev)
                    if kt >= 2:
                        issue_pv(kt - 2)
                for kt in range(max(0, nkt - 2), nkt):
                    issue_pv(kt)

                dens = [evac.tile([1, NQ], fp16, tag="den", name=f"den{_h}",
                                  bufs=2 * hpc)
                        for _h in range(hpc)]
                prev = (qt, pvs, dens)
                issue_dens(prev)

            issue_norm_rest(prev)

        def proj(b):
            for n in range(T // NQ):
                ns = n * NQ
                rt = xin.tile([P, KT_E * NQ], fp16, tag="xcol")
                nc.sync.dma_start(
                    rt[:].rearrange("p (k q) -> p k q", k=KT_E),
                    cc_outs[b][:, ns:ns + NQ].rearrange("(k p) q -> p k q",
                                                        p=P),
                )
                ps = stp.tile([P, NQ], f32, tag="st")
                for k in range(KT_E):
                    nc.tensor.matmul(
                        ps[0:OC, :],
                        lhsT=wp_sb[:, k * OC:(k + 1) * OC],
                        rhs=rt[:, k * NQ:(k + 1) * NQ],
                        start=(k == 0),
                        stop=(k == KT_E - 1),
                    )
                yo = evac.tile([OC, NQ], f32, tag="yo")
                nc.vector.tensor_scalar_add(yo[:], ps[0:OC, :], bp_sb[:, 0:1])
                nc.sync.dma_start(yT[:, b * T + ns:b * T + ns + NQ], yo[:])

        for b in range(B):
            attention(b)
            nc.gpsimd.collective_compute(
                "AllGather",
                mybir.AluOpType.bypass,
                replica_groups=[list(range(n_cores))],
                ins=[cc_ins[b][:].opt()],
                outs=[cc_outs[b][:].opt()],
            )
            if b >= 1:
                proj(b - 1)
        proj(B - 1)

    nc.compile()
    return nc


def shard_inputs(x, W_qkv, b_qkv, W_proj, b_proj, H, n_cores):
    B, T, C = x.shape
    hs = C // H
    hpc = H // n_cores
    CH = hpc * hs
    OC = C // n_cores
    x2 = np.asarray(x, dtype=np.float32).reshape(B * T, C)
    xT = np.ascontiguousarray(x2.T.astype(np.float16))
    W_qkv = np.asarray(W_qkv, dtype=np.float32)
    b_qkv = np.asarray(b_qkv, dtype=np.float32)
    W_proj = np.asarray(W_proj, dtype=np.float32)
    b_proj = np.asarray(b_proj, dtype=np.float32)
    in_maps = []
    for i in range(n_cores):
        sl = slice(i * CH, (i + 1) * CH)
        wqkv_i = np.ascontiguousarray(np.concatenate(
            [W_qkv[:, sl], W_qkv[:, C:][:, sl], W_qkv[:, 2 * C:][:, sl]],
            axis=1).astype(np.float16))
        bqkv_i = np.ascontiguousarray(np.stack(
            [b_qkv[sl], b_qkv[C:][sl], b_qkv[2 * C:][sl]], axis=1))
        wp_i = np.ascontiguousarray(
            W_proj[:, i * OC:(i + 1) * OC].astype(np.float16))
        bp_i = np.ascontiguousarray(b_proj[i * OC:(i + 1) * OC].reshape(OC, 1))
        in_maps.append({"xT": xT, "wqkv": wqkv_i, "bqkv": bqkv_i,
                        "wp": wp_i, "bp": bp_i})
    return in_maps


def gather_output(results, B, T, C, n_cores):
    yT = np.concatenate([results[i]["yT"] for i in range(n_cores)], axis=0)
    return np.ascontiguousarray(yT.T).reshape(B, T, C).astype(np.float32)


_NC_CACHE = {}


def _get_nc(B, T, C, H, n_cores):
    key = (B, T, C, H, n_cores)
    if key not in _NC_CACHE:
        _NC_CACHE[key] = build_attention_nc(B, T, C, H, n_cores)
    return _NC_CACHE[key]


def kernel(x, W_qkv, b_qkv, W_proj, b_proj):
    from concourse import bass_utils

    B, T, C = 4, 2048, 1024
    H, n_cores = 16, 8
    assert x.shape == (B, T, C)
    nc = _get_nc(B, T, C, H, n_cores)
    in_maps = shard_inputs(x, W_qkv, b_qkv, W_proj, b_proj, H, n_cores)
    res = bass_utils.run_bass_kernel_spmd(
        nc, in_maps, core_ids=list(range(n_cores))
    )
    return gather_output(res.results, B, T, C, n_cores)


# revision 15
# speedup vs baseline: 1.2718x; 1.1891x over previous
"""Causal self-attention on 8 Trainium2 NeuronCores (Bass/Tile).

Problem: nn_CausalSelfAttention (B=4, T=2048, C=1024, H=16 heads, fp32).

Sharding: tensor-parallel over heads for QKV projection + attention
(2 heads per core), per-batch AllGather of attention outputs (fp16,
transposed layout), then tensor-parallel over output columns for the
final projection (each core computes a 128-column slice of x@W_proj).

Schedule: software-pipelined per batch.  The attention kt-stream of
batch b is interleaved with "PE filler" groups — the QKV projection
row-tiles of batch b+1 and the output-projection row-tiles of batch
b-1 — so the tensor engine never idles while the scalar engine works
through the exp() stream (exp is the per-batch critical path; PE idle
gaps would also drop the HAM clock gate to half rate).

Layouts (feature dim on partitions everywhere):
  xT      [C, B*T]        input (fp16), replicated to all cores
  Q^T,K^T [CH, B*T]       CH = 2 heads x 64, heads stacked on
                          partitions 0:64 / 64:128
  V       [B*T, CH]       matmul lhsT for P@V, stored tiled with an
                          extra ones-column per head so the PV matmul
                          also produces softmax denominators
  S^T     [kr, 2*q] pair  scores transposed, both heads side by side in
                          one 2-bank PSUM tile; the two score matmuls
                          are K=64 row-tiled pairs (tile_position
                          (0,0)/(64,0)) that execute concurrently; one
                          exp() activation covers both heads
  attn^T  [CH, T] fp16    per-core, per-batch -> AllGather -> [C, T]
  y^T     [OC, B*T]       per-core 128-column slice of the final output

Softmax: unnormalized exp (scores are O(1)); causal mask = PE add of a
-60000 upper-triangular [128,128] constant onto the diagonal strip;
denominator from the V ones-column; division via
reciprocal_approx_fast on the PE-broadcast denominator.
All matmuls run fp16 inputs with fp32 PSUM accumulation.
"""

import numpy as np
from contextlib import ExitStack

P = 128
NQ = 512  # q/moving-operand tile width
MASKVAL = -60000.0


def build_attention_nc(B, T, C, H, n_cores):
    import concourse.bass as bass  # noqa: F401
    import concourse.bacc as bacc
    import concourse.tile as tile
    import concourse.mybir as mybir

    f32 = mybir.dt.float32
    fp16 = mybir.dt.float16
    Exp = mybir.ActivationFunctionType.Exp

    hs = C // H              # head size
    hpc = H // n_cores       # heads per core
    CH = hpc * hs            # qkv channels per core
    OC = C // n_cores        # output columns per core
    NT = B * T               # tokens
    KT_E = C // P            # contraction tiles over embedding dim
    TQ = T // NQ             # q tiles per batch
    TK = T // P              # kr tiles per batch
    TKALL = NT // P          # kr tiles over all batches
    DPB = NQ // P            # kr-tiles crossing one q-tile's diagonal
    WV = hpc * (hs + 1)      # V storage width per kr-tile (with ones cols)

    assert T % NQ == 0 and C % P == 0 and NT % NQ == 0
    assert CH == P and H % n_cores == 0 and hpc == 2 and hs == 64
    scale = 1.0 / float(np.sqrt(hs))

    nc = bacc.Bacc("TRN2", target_bir_lowering=False, debug=False,
                   num_devices=n_cores)

    xT = nc.dram_tensor("xT", [C, NT], fp16, kind="ExternalInput")
    wqkv = nc.dram_tensor("wqkv", [C, 3 * CH], fp16, kind="ExternalInput")
    bqkv = nc.dram_tensor("bqkv", [CH, 3], f32, kind="ExternalInput")
    wp = nc.dram_tensor("wp", [C, OC], fp16, kind="ExternalInput")
    bp = nc.dram_tensor("bp", [OC, 1], f32, kind="ExternalInput")
    yT = nc.dram_tensor("yT", [OC, NT], f32, kind="ExternalOutput")

    ident_np = np.eye(P, dtype=np.float16)
    # mask[p, c] = MASKVAL where kr-offset p > q-offset c (strictly lower)
    mask_np = np.where(
        np.arange(P)[:, None] > np.arange(P)[None, :],
        np.float16(MASKVAL), np.float16(0.0)).astype(np.float16)
    ident_dram = nc.inline_tensor(ident_np, name="ident_const")
    mask_dram = nc.inline_tensor(mask_np, name="mask_const")
    ones_dram = nc.inline_tensor(np.ones((P, hs), dtype=np.float16),
                                 name="ones_const")
    vones_dram = nc.inline_tensor(np.ones((P, TKALL * hpc), dtype=np.float16),
                                  name="vones_const")

    with tile.TileContext(nc) as tc, ExitStack() as ctx:
        const = ctx.enter_context(tc.tile_pool(name="const", bufs=1))
        big = ctx.enter_context(tc.tile_pool(name="big", bufs=1))
        xin = ctx.enter_context(tc.tile_pool(name="xin", bufs=6))
        evac = ctx.enter_context(tc.tile_pool(name="evac", bufs=3))
        pexp = ctx.enter_context(tc.tile_pool(name="pexp", bufs=4))
        stp = ctx.enter_context(tc.tile_pool(name="stp", bufs=3, space="PSUM"))
        pvp = ctx.enter_context(tc.tile_pool(name="pvp", bufs=2, space="PSUM"))
        dram = ctx.enter_context(tc.tile_pool(name="dram", bufs=1, space="DRAM"))

        ident_t = const.tile([P, P], fp16)
        mask_sb = const.tile([P, P], fp16)
        ones_sb = const.tile([P, hs], fp16)
        bqkv_sb = const.tile([CH, 3], f32)
        bp_sb = const.tile([OC, 1], f32)
        w_sb = const.tile([P, KT_E * 3 * CH], fp16)
        wp_sb = const.tile([P, KT_E * OC], fp16)

        nc.sync.dma_start(ident_t[:], ident_dram[:])
        nc.sync.dma_start(mask_sb[:], mask_dram[:])
        nc.sync.dma_start(ones_sb[:], ones_dram[:])
        nc.sync.dma_start(bqkv_sb[:], bqkv[:])
        nc.sync.dma_start(bp_sb[:], bp[:])
        # weights k-slice by k-slice so the first QKV matmul can start
        # ~1us after the first two transfers instead of after the full load
        for k in range(KT_E):
            nc.sync.dma_start(
                w_sb[:, k * 3 * CH:(k + 1) * 3 * CH],
                wqkv[k * P:(k + 1) * P, :],
            )
        nc.sync.dma_start(
            wp_sb[:].rearrange("p (k m) -> p k m", k=KT_E),
            wp[:].rearrange("(k p) m -> p k m", p=P),
        )

        QT = big.tile([P, NT], fp16)
        KTp = big.tile([P, NT], fp16)
        VT = big.tile([CH, NT], fp16)
        Vn = big.tile([P, TKALL * WV], fp16)

        # ones columns of V (softmax denominator trick)
        ones_view = Vn[:].rearrange("p (v h d) -> p v h d", h=hpc, d=hs + 1)[
            :, :, :, hs:hs + 1
        ]
        nc.sync.dma_start(
            ones_view,
            vones_dram[:].rearrange("p (v h d) -> p v h d", h=hpc, d=1),
        )

        # ---- QKV projection for one 512-token row-tile; emitted either
        # directly (batch 0) or as filler groups inside an attention batch
        def qkv_row_fillers(n):
            ns = n * NQ
            xt_box = []

            def load_x():
                xt = xin.tile([P, KT_E * NQ], fp16, tag="xcol")
                nc.sync.dma_start(
                    xt[:].rearrange("p (k q) -> p k q", k=KT_E),
                    xT[:, ns:ns + NQ].rearrange("(k p) q -> p k q", p=P),
                )
                xt_box.append(xt)

            def mm_group(m):
                def go():
                    xt = xt_box[0]
                    ps = stp.tile([P, 2 * NQ], f32, tag="st")
                    for k in range(KT_E):
                        nc.tensor.matmul(
                            ps[:, 0:NQ],
                            lhsT=w_sb[:, k * 3 * CH + m * CH:
                                      k * 3 * CH + (m + 1) * CH],
                            rhs=xt[:, k * NQ:(k + 1) * NQ],
                            start=(k == 0),
                            stop=(k == KT_E - 1),
                        )
                    dst = (QT, KTp, VT)[m]
                    nc.vector.tensor_scalar_add(dst[:, ns:ns + NQ],
                                                ps[:, 0:NQ],
                                                bqkv_sb[:, m:m + 1])
                return go

            def transposes():
                tp = stp.tile([P, DPB * CH], fp16, tag="st", name="tp")
                for j in range(DPB):
                    nc.tensor.transpose(
                        tp[:, j * CH:(j + 1) * CH],
                        VT[:, (n * DPB + j) * P:(n * DPB + j + 1) * P],
                        ident_t[:],
                    )
                vi0 = n * DPB
                dst = Vn[:, vi0 * WV:(vi0 + DPB) * WV].rearrange(
                    "p (v h d) -> p v h d", h=hpc, d=hs + 1
                )[:, :, :, 0:hs]
                nc.vector.tensor_copy(dst, tp[:].rearrange(
                    "p (v h d) -> p v h d", h=hpc, d=hs))

            load_x()
            return [mm_group(0), mm_group(1), mm_group(2), transposes]

        # ---- per-batch AllGather buffers (batch B-1 split in halves to
        # shrink the tail)
        cc_ins, cc_outs = [], []
        for b in range(B):
            nh = 2 if b == B - 1 else 1
            cc_ins.append([dram.tile([CH, T // nh], fp16,
                                     name=f"ccin{b}_{h}") for h in range(nh)])
            cc_outs.append([dram.tile([n_cores * CH, T // nh], fp16,
                                      addr_space="Shared",
                                      name=f"ccout{b}_{h}")
                            for h in range(nh)])

        def issue_ag(b, half, nh):
            nc.gpsimd.collective_compute(
                "AllGather",
                mybir.AluOpType.bypass,
                replica_groups=[list(range(n_cores))],
                ins=[cc_ins[b][half][:].opt()],
                outs=[cc_outs[b][half][:].opt()],
            )

        def proj_row_fillers(b):
            # output projection of batch b, one filler per 512-token row
            nh = len(cc_outs[b])
            hw = T // nh

            def row(n):
                def go():
                    ns = n * NQ
                    rt = xin.tile([P, KT_E * NQ], fp16, tag="xcol")
                    # issue on the ACT HWDGE queue so a wait on the
                    # AllGather doesn't head-of-line block the sync queue
                    nc.scalar.dma_start(
                        rt[:].rearrange("p (k q) -> p k q", k=KT_E),
                        cc_outs[b][ns // hw][:, ns % hw:ns % hw + NQ]
                        .rearrange("(k p) q -> p k q", p=P),
                    )
                    ps = stp.tile([P, 2 * NQ], f32, tag="st")
                    for k in range(KT_E):
                        nc.tensor.matmul(
                            ps[0:OC, 0:NQ],
                            lhsT=wp_sb[:, k * OC:(k + 1) * OC],
                            rhs=rt[:, k * NQ:(k + 1) * NQ],
                            start=(k == 0),
                            stop=(k == KT_E - 1),
                        )
                    yo = evac.tile([OC, NQ], f32, tag="yo")
                    nc.vector.tensor_scalar_add(yo[:], ps[0:OC, 0:NQ],
                                                bp_sb[:, 0:1])
                    nc.sync.dma_start(yT[:, b * T + ns:b * T + ns + NQ],
                                      yo[:])
                return go

            return [row(n) for n in range(T // NQ)]

        def attention(b, fillers):
            """kt-stream of batch b with PE filler groups interleaved."""
            fillers = list(fillers)

            def drain(k=1):
                for _ in range(k):
                    if fillers:
                        f = fillers.pop(0)
                        if f is not None:
                            f()

            nh = len(cc_ins[b])
            hw = T // nh

            for qt in range(TQ):
                qs = b * T + qt * NQ
                nkt = DPB * qt + DPB
                pvs = [pvp.tile([P, NQ], f32, tag="pv", name=f"pv{_h}")
                       for _h in range(hpc)]
                pes = {}

                def issue_st(kt, qt=qt, qs=qs, pes=pes):
                    ks = b * T + kt * P
                    diag = kt >= DPB * qt
                    j = kt - DPB * qt
                    c0 = j * P if diag else 0
                    st = stp.tile([P, 2 * NQ], f32, tag="st")
                    for hh in range(hpc):
                        nc.tensor.matmul(
                            st[:, hh * NQ + c0:(hh + 1) * NQ],
                            lhsT=KTp[hh * hs:(hh + 1) * hs, ks:ks + P],
                            rhs=QT[hh * hs:(hh + 1) * hs, qs + c0:qs + NQ],
                            start=True,
                            stop=not diag,
                            tile_position=(hh * hs, 0),
                        )
                    if diag:
                        for hh in range(hpc):
                            nc.tensor.matmul(
                                st[:, hh * NQ + c0:hh * NQ + c0 + P],
                                lhsT=ident_t[:],
                                rhs=mask_sb[:],
                                start=False,
                                stop=True,
                            )
                    pe_t = pexp.tile([P, 2 * NQ], fp16, tag="pe")
                    nc.scalar.activation(
                        pe_t[:].rearrange("p (h q) -> p h q", h=hpc)
                        [:, :, c0:NQ],
                        st[:].rearrange("p (h q) -> p h q", h=hpc)
                        [:, :, c0:NQ],
                        Exp, scale=scale)
                    pes[kt] = (pe_t, c0)

                def issue_pv(kt, nkt=nkt, pvs=pvs, pes=pes):
                    vi = b * TK + kt
                    pe_t, c0 = pes.pop(kt)
                    for hh in range(hpc):
                        nc.tensor.matmul(
                            pvs[hh][0:hs + 1, c0:NQ],
                            lhsT=Vn[:, vi * WV + hh * (hs + 1):
                                    vi * WV + (hh + 1) * (hs + 1)],
                            rhs=pe_t[:, hh * NQ + c0:(hh + 1) * NQ],
                            start=(kt == 0),
                            stop=(kt == nkt - 1),
                        )

                for kt in range(nkt):
                    issue_st(kt)
                    if kt % 2 == 1:
                        drain()
                    if kt >= 2:
                        issue_pv(kt - 2)
                for kt in range(max(0, nkt - 2), nkt):
                    issue_pv(kt)

                # normalization: denominators -> broadcast -> reciprocal ->
                # scale; a filler between the DVE dens copy and the PE
                # broadcast hides the DVE latency
                dens = evac.tile([1, 2 * NQ], fp16, tag="den", bufs=3)
                for hh in range(hpc):
                    nc.vector.tensor_copy(dens[:, hh * NQ:(hh + 1) * NQ],
                                          pvs[hh][hs:hs + 1, :])
                drain()
                bc = stp.tile([P, 2 * NQ], f32, tag="st")
                for hh in range(hpc):
                    nc.tensor.matmul(
                        bc[0:hs, hh * NQ:(hh + 1) * NQ],
                        lhsT=ones_sb[0:1, :],
                        rhs=dens[:, hh * NQ:(hh + 1) * NQ],
                        start=True,
                        stop=True,
                    )
                recs = evac.tile([hs, 2 * NQ], f32, tag="rec", bufs=2)
                nc.vector.reciprocal_approx_fast(recs[:], bc[0:hs, :])
                for hh in range(hpc):
                    ao = evac.tile([hs, NQ], fp16, tag="ao")
                    nc.vector.tensor_mul(ao[:], pvs[hh][0:hs, :],
                                         recs[:, hh * NQ:(hh + 1) * NQ])
                    qcol = qt * NQ
                    nc.sync.dma_start(
                        cc_ins[b][qcol // hw][hh * hs:(hh + 1) * hs,
                                              qcol % hw:qcol % hw + NQ],
                        ao[:],
                    )
                if nh == 2 and qt == TQ // 2 - 1:
                    issue_ag(b, 0, nh)

            issue_ag(b, nh - 1, nh)
            # leftover fillers (also give the last AG time to land before
            # the dependent projection rows run)
            while fillers:
                drain()

        # ---- main schedule ----
        for g in qkv_row_fillers(0) + qkv_row_fillers(1) + \
                qkv_row_fillers(2) + qkv_row_fillers(3):
            g()
        for b in range(B):
            fillers = []
            if b + 1 < B:
                for n in range(TQ):
                    fillers += qkv_row_fillers((b + 1) * TQ + n)
            else:
                # keep the projection of b-1 late in the stream so its
                # AllGather has completed by the time the PE reaches it
                fillers += [None] * 16
            if b >= 1:
                fillers += proj_row_fillers(b - 1)
            attention(b, fillers)
        proj_last = proj_row_fillers(B - 1)
        for g in proj_last:
            g()

    nc.compile()
    return nc


def shard_inputs(x, W_qkv, b_qkv, W_proj, b_proj, H, n_cores):
    B, T, C = x.shape
    hs = C // H
    hpc = H // n_cores
    CH = hpc * hs
    OC = C // n_cores
    x2 = np.asarray(x, dtype=np.float32).reshape(B * T, C)
    xT = np.ascontiguousarray(x2.T.astype(np.float16))
    W_qkv = np.asarray(W_qkv, dtype=np.float32)
    b_qkv = np.asarray(b_qkv, dtype=np.float32)
    W_proj = np.asarray(W_proj, dtype=np.float32)
    b_proj = np.asarray(b_proj, dtype=np.float32)
    in_maps = []
    for i in range(n_cores):
        sl = slice(i * CH, (i + 1) * CH)
        wqkv_i = np.ascontiguousarray(np.concatenate(
            [W_qkv[:, sl], W_qkv[:, C:][:, sl], W_qkv[:, 2 * C:][:, sl]],
            axis=1).astype(np.float16))
        bqkv_i = np.ascontiguousarray(np.stack(
            [b_qkv[sl], b_qkv[C:][sl], b_qkv[2 * C:][sl]], axis=1))
        wp_i = np.ascontiguousarray(
            W_proj[:, i * OC:(i + 1) * OC].astype(np.float16))
        bp_i = np.ascontiguousarray(b_proj[i * OC:(i + 1) * OC].reshape(OC, 1))
        in_maps.append({"xT": xT, "wqkv": wqkv_i, "bqkv": bqkv_i,
                        "wp": wp_i, "bp": bp_i})
    return in_maps


def gather_output(results, B, T, C, n_cores):
    yT = np.concatenate([results[i]["yT"] for i in range(n_cores)], axis=0)
    return np.ascontiguousarray(yT.T).reshape(B, T, C).astype(np.float32)


_NC_CACHE = {}


def _get_nc(B, T, C, H, n_cores):
    key = (B, T, C, H, n_cores)
    if key not in _NC_CACHE:
        _NC_CACHE[key] = build_attention_nc(B, T, C, H, n_cores)
    return _NC_CACHE[key]


def kernel(x, W_qkv, b_qkv, W_proj, b_proj):
    from concourse import bass_utils

    B, T, C = 4, 2048, 1024
    H, n_cores = 16, 8
    assert x.shape == (B, T, C)
    nc = _get_nc(B, T, C, H, n_cores)
    in_maps = shard_inputs(x, W_qkv, b_qkv, W_proj, b_proj, H, n_cores)
    res = bass_utils.run_bass_kernel_spmd(
        nc, in_maps, core_ids=list(range(n_cores))
    )
    return gather_output(res.results, B, T, C, n_cores)


# revision 24
# speedup vs baseline: 1.3480x; 1.0599x over previous
"""Causal self-attention on 8 Trainium2 NeuronCores (Bass/Tile).

Problem: nn_CausalSelfAttention (B=4, T=2048, C=1024, H=16 heads, fp32).

Sharding: tensor-parallel over heads for QKV projection + attention
(2 heads per core), per-batch AllGather of attention outputs (fp16,
transposed layout), then tensor-parallel over output columns for the
final projection (each core computes a 128-column slice of x@W_proj).

Schedule: software-pipelined per batch.  The attention kt-stream of
batch b is interleaved with "PE filler" groups — the QKV projection
row-tiles of batch b+1 and the output-projection row-tiles of batch
b-1 — so the tensor engine never idles while the scalar engine works
through the exp() stream (exp is the per-batch critical path; PE idle
gaps would also drop the HAM clock gate to half rate).

Layouts (feature dim on partitions everywhere):
  xT      [C, B*T]        input (fp16), replicated to all cores
  Q^T,K^T [CH, B*T]       CH = 2 heads x 64, heads stacked on
                          partitions 0:64 / 64:128
  V       [B*T, CH]       matmul lhsT for P@V, stored tiled with an
                          extra ones-column per head so the PV matmul
                          also produces softmax denominators
  S^T     [kr, 2*q] pair  scores transposed, both heads side by side in
                          one 2-bank PSUM tile; the two score matmuls
                          are K=64 row-tiled pairs (tile_position
                          (0,0)/(64,0)) that execute concurrently; one
                          exp() activation covers both heads
  attn^T  [CH, T] fp16    per-core, per-batch -> AllGather -> [C, T]
  y^T     [OC, B*T]       per-core 128-column slice of the final output

Softmax: unnormalized exp (scores are O(1)); causal mask = PE add of a
-60000 upper-triangular [128,128] constant onto the diagonal strip;
denominator from the V ones-column; division via
reciprocal_approx_fast on the PE-broadcast denominator.
All matmuls run fp16 inputs with fp32 PSUM accumulation.
"""

import numpy as np
from contextlib import ExitStack

P = 128
NQ = 512  # q/moving-operand tile width
MASKVAL = -60000.0


def build_attention_nc(B, T, C, H, n_cores):
    import concourse.bass as bass  # noqa: F401
    import concourse.bacc as bacc
    import concourse.tile as tile
    import concourse.mybir as mybir

    f32 = mybir.dt.float32
    fp16 = mybir.dt.float16
    Exp = mybir.ActivationFunctionType.Exp

    hs = C // H              # head size
    hpc = H // n_cores       # heads per core
    CH = hpc * hs            # qkv channels per core
    OC = C // n_cores        # output columns per core
    NT = B * T               # tokens
    KT_E = C // P            # contraction tiles over embedding dim
    TQ = T // NQ             # q tiles per batch
    TK = T // P              # kr tiles per batch
    TKALL = NT // P          # kr tiles over all batches
    DPB = NQ // P            # kr-tiles crossing one q-tile's diagonal
    WV = hpc * (hs + 1)      # V storage width per kr-tile (with ones cols)

    assert T % NQ == 0 and C % P == 0 and NT % NQ == 0
    assert CH == P and H % n_cores == 0 and hpc == 2 and hs == 64
    scale = 1.0 / float(np.sqrt(hs))

    nc = bacc.Bacc("TRN2", target_bir_lowering=False, debug=False,
                   num_devices=n_cores)

    xT = nc.dram_tensor("xT", [C, NT], fp16, kind="ExternalInput")
    wqkv = nc.dram_tensor("wqkv", [C, 3 * CH], fp16, kind="ExternalInput")
    bqkv = nc.dram_tensor("bqkv", [CH, 3], f32, kind="ExternalInput")
    wp = nc.dram_tensor("wp", [C, OC], fp16, kind="ExternalInput")
    bp = nc.dram_tensor("bp", [OC, 1], f32, kind="ExternalInput")
    yT = nc.dram_tensor("yT", [OC, NT], f32, kind="ExternalOutput")

    ident_np = np.eye(P, dtype=np.float16)
    # mask[p, c] = MASKVAL where kr-offset p > q-offset c (strictly lower)
    mask_np = np.where(
        np.arange(P)[:, None] > np.arange(P)[None, :],
        np.float16(MASKVAL), np.float16(0.0)).astype(np.float16)
    ident_dram = nc.inline_tensor(ident_np, name="ident_const")
    mask_dram = nc.inline_tensor(mask_np, name="mask_const")
    ones_dram = nc.inline_tensor(np.ones((P, hs), dtype=np.float16),
                                 name="ones_const")
    # Vn image with the denominator ones-columns baked in; the value
    # columns are overwritten by the V transposes at runtime
    vinit_np = np.zeros((P, TKALL * WV), dtype=np.float16)
    vinit_np.reshape(P, TKALL, hpc, hs + 1)[:, :, :, hs] = 1.0
    vinit_dram = nc.inline_tensor(vinit_np, name="vinit_const")

    with tile.TileContext(nc) as tc, ExitStack() as ctx:
        const = ctx.enter_context(tc.tile_pool(name="const", bufs=1))
        big = ctx.enter_context(tc.tile_pool(name="big", bufs=1))
        xin = ctx.enter_context(tc.tile_pool(name="xin", bufs=6))
        evac = ctx.enter_context(tc.tile_pool(name="evac", bufs=3))
        pexp = ctx.enter_context(tc.tile_pool(name="pexp", bufs=4))
        stp = ctx.enter_context(tc.tile_pool(name="stp", bufs=3, space="PSUM"))
        pvp = ctx.enter_context(tc.tile_pool(name="pvp", bufs=2, space="PSUM"))
        dram = ctx.enter_context(tc.tile_pool(name="dram", bufs=1, space="DRAM"))

        ident_t = const.tile([P, P], fp16)
        mask_sb = const.tile([P, P], fp16)
        ones_sb = const.tile([P, hs], fp16)
        bqkv_sb = const.tile([CH, 3], f32)
        bp_sb = const.tile([OC, 1], f32)
        w_sb = const.tile([P, KT_E * 3 * CH], fp16)
        wp_sb = const.tile([P, KT_E * OC], fp16)

        nc.sync.dma_start(bqkv_sb[:], bqkv[:])
        nc.sync.dma_start(ident_t[:], ident_dram[:])
        nc.sync.dma_start(mask_sb[:], mask_dram[:])
        nc.sync.dma_start(ones_sb[:], ones_dram[:])
        nc.sync.dma_start(bp_sb[:], bp[:])
        # weights k-slice by k-slice so the first QKV matmul can start
        # ~1us after the first two transfers instead of after the full load
        for k in range(KT_E):
            nc.sync.dma_start(
                w_sb[:, k * 3 * CH:(k + 1) * 3 * CH],
                wqkv[k * P:(k + 1) * P, :],
            )

        def load_late_consts():
            # not needed until the projection phase
            nc.sync.dma_start(
                wp_sb[:].rearrange("p (k m) -> p k m", k=KT_E),
                wp[:].rearrange("(k p) m -> p k m", p=P),
            )

        QT = big.tile([P, NT], fp16)
        KTp = big.tile([P, NT], fp16)
        VT = big.tile([CH, NT], fp16)
        Vn = big.tile([P, TKALL * WV], fp16)

        # ones columns of V (softmax denominator trick) via one contiguous
        # DMA of the full Vn image; value columns get overwritten later
        nc.sync.dma_start(Vn[:], vinit_dram[:])

        # ---- QKV projection for one 512-token row-tile; emitted either
        # directly (batch 0) or as filler groups inside an attention batch
        def qkv_row_fillers(n):
            ns = n * NQ
            xt_box = []

            def load_x():
                xt = xin.tile([P, KT_E * NQ], fp16, tag="xcol")
                nc.sync.dma_start(
                    xt[:].rearrange("p (k q) -> p k q", k=KT_E),
                    xT[:, ns:ns + NQ].rearrange("(k p) q -> p k q", p=P),
                )
                xt_box.append(xt)

            def mm_group(m):
                def go():
                    xt = xt_box[0]
                    ps = stp.tile([P, 2 * NQ], f32, tag="st")
                    for k in range(KT_E):
                        nc.tensor.matmul(
                            ps[:, 0:NQ],
                            lhsT=w_sb[:, k * 3 * CH + m * CH:
                                      k * 3 * CH + (m + 1) * CH],
                            rhs=xt[:, k * NQ:(k + 1) * NQ],
                            start=(k == 0),
                            stop=(k == KT_E - 1),
                        )
                    dst = (QT, KTp, VT)[m]
                    nc.vector.tensor_scalar_add(dst[:, ns:ns + NQ],
                                                ps[:, 0:NQ],
                                                bqkv_sb[:, m:m + 1])
                return go

            def transposes():
                tp = stp.tile([P, DPB * CH], fp16, tag="st", name="tp")
                for j in range(DPB):
                    nc.tensor.transpose(
                        tp[:, j * CH:(j + 1) * CH],
                        VT[:, (n * DPB + j) * P:(n * DPB + j + 1) * P],
                        ident_t[:],
                    )
                vi0 = n * DPB
                dst = Vn[:, vi0 * WV:(vi0 + DPB) * WV].rearrange(
                    "p (v h d) -> p v h d", h=hpc, d=hs + 1
                )[:, :, :, 0:hs]
                nc.vector.tensor_copy(dst, tp[:].rearrange(
                    "p (v h d) -> p v h d", h=hpc, d=hs))

            load_x()
            return [mm_group(0), mm_group(1), mm_group(2), transposes]

        # ---- per-batch AllGather pieces: every batch is gathered in two
        # halves so the projection rows unblock early; the last batch's
        # second half is further split per q-tile to shrink the tail
        pieces = []  # per batch: list of (start_qt, n_qt)
        for b in range(B):
            if b == B - 1:
                pieces.append([(0, 2), (2, 1), (3, 1)])
            else:
                pieces.append([(0, 2), (2, 2)])
        cc_ins, cc_outs = [], []
        for b in range(B):
            cc_ins.append([dram.tile([CH, n * NQ], fp16,
                                     name=f"ccin{b}_{q0}")
                           for (q0, n) in pieces[b]])
            cc_outs.append([dram.tile([n_cores * CH, n * NQ], fp16,
                                      addr_space="Shared",
                                      name=f"ccout{b}_{q0}")
                            for (q0, n) in pieces[b]])

        def piece_of(b, qt):
            for i, (q0, n) in enumerate(pieces[b]):
                if q0 <= qt < q0 + n:
                    return i, (qt - q0) * NQ
            raise AssertionError

        def issue_ag(b, i):
            nc.gpsimd.collective_compute(
                "AllGather",
                mybir.AluOpType.bypass,
                replica_groups=[list(range(n_cores))],
                ins=[cc_ins[b][i][:].opt()],
                outs=[cc_outs[b][i][:].opt()],
            )

        def proj_row_fillers(b):
            # output projection of batch b, one filler per 512-token row
            def row(n):
                def go():
                    i, off = piece_of(b, n)
                    rt = xin.tile([P, KT_E * NQ], fp16, tag="xcol")
                    # issue on the ACT HWDGE queue so a wait on the
                    # AllGather doesn't head-of-line block the sync queue
                    nc.scalar.dma_start(
                        rt[:].rearrange("p (k q) -> p k q", k=KT_E),
                        cc_outs[b][i][:, off:off + NQ]
                        .rearrange("(k p) q -> p k q", p=P),
                    )
                    ps = stp.tile([P, 2 * NQ], f32, tag="st")
                    for k in range(KT_E):
                        nc.tensor.matmul(
                            ps[0:OC, 0:NQ],
                            lhsT=wp_sb[:, k * OC:(k + 1) * OC],
                            rhs=rt[:, k * NQ:(k + 1) * NQ],
                            start=(k == 0),
                            stop=(k == KT_E - 1),
                        )
                    yo = evac.tile([OC, NQ], f32, tag="yo")
                    nc.vector.tensor_scalar_add(yo[:], ps[0:OC, 0:NQ],
                                                bp_sb[:, 0:1])
                    nc.sync.dma_start(yT[:, b * T + n * NQ:
                                         b * T + n * NQ + NQ], yo[:])
                return go

            return [row(n) for n in range(T // NQ)]

        def attention(b, fillers):
            """kt-stream of batch b with PE filler groups interleaved."""
            fillers = list(fillers)

            def drain(k=1):
                for _ in range(k):
                    if fillers:
                        f = fillers.pop(0)
                        if f is not None:
                            f()

            for qt in range(TQ):
                qs = b * T + qt * NQ
                nkt = DPB * qt + DPB
                pvs = [pvp.tile([P, NQ], f32, tag="pv", name=f"pv{_h}")
                       for _h in range(hpc)]
                pes = {}

                def issue_st(kt, qt=qt, qs=qs, pes=pes):
                    ks = b * T + kt * P
                    diag = kt >= DPB * qt
                    j = kt - DPB * qt
                    c0 = j * P if diag else 0
                    st = stp.tile([P, 2 * NQ], f32, tag="st")
                    for hh in range(hpc):
                        nc.tensor.matmul(
                            st[:, hh * NQ + c0:(hh + 1) * NQ],
                            lhsT=KTp[hh * hs:(hh + 1) * hs, ks:ks + P],
                            rhs=QT[hh * hs:(hh + 1) * hs, qs + c0:qs + NQ],
                            start=True,
                            stop=not diag,
                            tile_position=(hh * hs, 0),
                        )
                    if diag:
                        for hh in range(hpc):
                            nc.tensor.matmul(
                                st[:, hh * NQ + c0:hh * NQ + c0 + P],
                                lhsT=ident_t[:],
                                rhs=mask_sb[:],
                                start=False,
                                stop=True,
                            )
                    pe_t = pexp.tile([P, 2 * NQ], fp16, tag="pe")
                    nc.scalar.activation(
                        pe_t[:].rearrange("p (h q) -> p h q", h=hpc)
                        [:, :, c0:NQ],
                        st[:].rearrange("p (h q) -> p h q", h=hpc)
                        [:, :, c0:NQ],
                        Exp, scale=scale)
                    pes[kt] = (pe_t, c0)

                def issue_pv(kt, nkt=nkt, pvs=pvs, pes=pes):
                    vi = b * TK + kt
                    pe_t, c0 = pes.pop(kt)
                    for hh in range(hpc):
                        nc.tensor.matmul(
                            pvs[hh][0:hs + 1, c0:NQ],
                            lhsT=Vn[:, vi * WV + hh * (hs + 1):
                                    vi * WV + (hh + 1) * (hs + 1)],
                            rhs=pe_t[:, hh * NQ + c0:(hh + 1) * NQ],
                            start=(kt == 0),
                            stop=(kt == nkt - 1),
                        )

                for kt in range(nkt):
                    issue_st(kt)
                    if kt % 2 == 1:
                        drain()
                    if kt >= 2:
                        issue_pv(kt - 2)
                for kt in range(max(0, nkt - 2), nkt):
                    issue_pv(kt)

                # normalization: denominators -> broadcast -> reciprocal ->
                # scale; a filler between the DVE dens copy and the PE
                # broadcast hides the DVE latency
                dens = evac.tile([1, 2 * NQ], fp16, tag="den", bufs=3)
                for hh in range(hpc):
                    nc.vector.tensor_copy(dens[:, hh * NQ:(hh + 1) * NQ],
                                          pvs[hh][hs:hs + 1, :])
                drain()
                bc = stp.tile([P, 2 * NQ], f32, tag="st")
                for hh in range(hpc):
                    nc.tensor.matmul(
                        bc[0:hs, hh * NQ:(hh + 1) * NQ],
                        lhsT=ones_sb[0:1, :],
                        rhs=dens[:, hh * NQ:(hh + 1) * NQ],
                        start=True,
                        stop=True,
                    )
                recs = evac.tile([hs, 2 * NQ], f32, tag="rec", bufs=2)
                nc.vector.reciprocal_approx_fast(recs[:], bc[0:hs, :])
                pi, off = piece_of(b, qt)
                for hh in range(hpc):
                    ao = evac.tile([hs, NQ], fp16, tag="ao")
                    nc.vector.tensor_mul(ao[:], pvs[hh][0:hs, :],
                                         recs[:, hh * NQ:(hh + 1) * NQ])
                    nc.sync.dma_start(
                        cc_ins[b][pi][hh * hs:(hh + 1) * hs, off:off + NQ],
                        ao[:],
                    )
                if qt == pieces[b][pi][0] + pieces[b][pi][1] - 1:
                    issue_ag(b, pi)
            # leftover fillers (also give the last AG time to land before
            # the dependent projection rows run)
            while fillers:
                drain()

        # ---- main schedule ----
        row0 = qkv_row_fillers(0)
        row0[0]()
        load_late_consts()
        for g in row0[1:]:
            g()
        for g in qkv_row_fillers(1) + qkv_row_fillers(2) + qkv_row_fillers(3):
            g()
        proj_last = proj_row_fillers(B - 1)
        for b in range(B):
            if b + 1 < B:
                qkv = []
                for n in range(TQ):
                    qkv += qkv_row_fillers((b + 1) * TQ + n)
            else:
                qkv = [None] * 16
            # proj rows of b-1: rows 0,1 depend on the first AG piece of
            # b-1 (issued mid-attention(b-1), long done); rows 2,3 on the
            # second piece (issued at attention(b-1) end) -> place late
            proj = proj_row_fillers(b - 1) if b >= 1 else [None] * 4
            fillers = qkv[0:8] + proj[0:2] + qkv[8:16] + proj[2:4]
            if b + 1 == B:
                # last batch: slot its own first-piece projection rows at
                # the very end of the stream (their AG is issued at qt1)
                fillers += proj_last[0:2]
            attention(b, fillers)
        # tail: the remaining projection rows of the last batch, each
        # gated by its own per-q-tile AllGather piece
        for g in proj_last[2:4]:
            g()

    nc.compile()
    return nc


def shard_inputs(x, W_qkv, b_qkv, W_proj, b_proj, H, n_cores):
    B, T, C = x.shape
    hs = C // H
    hpc = H // n_cores
    CH = hpc * hs
    OC = C // n_cores
    x2 = np.asarray(x, dtype=np.float32).reshape(B * T, C)
    xT = np.ascontiguousarray(x2.T.astype(np.float16))
    W_qkv = np.asarray(W_qkv, dtype=np.float32)
    b_qkv = np.asarray(b_qkv, dtype=np.float32)
    W_proj = np.asarray(W_proj, dtype=np.float32)
    b_proj = np.asarray(b_proj, dtype=np.float32)
    in_maps = []
    for i in range(n_cores):
        sl = slice(i * CH, (i + 1) * CH)
        wqkv_i = np.ascontiguousarray(np.concatenate(
            [W_qkv[:, sl], W_qkv[:, C:][:, sl], W_qkv[:, 2 * C:][:, sl]],
            axis=1).astype(np.float16))
        bqkv_i = np.ascontiguousarray(np.stack(
            [b_qkv[sl], b_qkv[C:][sl], b_qkv[2 * C:][sl]], axis=1))
        wp_i = np.ascontiguousarray(
            W_proj[:, i * OC:(i + 1) * OC].astype(np.float16))
        bp_i = np.ascontiguousarray(b_proj[i * OC:(i + 1) * OC].reshape(OC, 1))
        in_maps.append({"xT": xT, "wqkv": wqkv_i, "bqkv": bqkv_i,
                        "wp": wp_i, "bp": bp_i})
    return in_maps


def gather_output(results, B, T, C, n_cores):
    yT = np.concatenate([results[i]["yT"] for i in range(n_cores)], axis=0)
    return np.ascontiguousarray(yT.T).reshape(B, T, C).astype(np.float32)


_NC_CACHE = {}


def _get_nc(B, T, C, H, n_cores):
    key = (B, T, C, H, n_cores)
    if key not in _NC_CACHE:
        _NC_CACHE[key] = build_attention_nc(B, T, C, H, n_cores)
    return _NC_CACHE[key]


def kernel(x, W_qkv, b_qkv, W_proj, b_proj):
    from concourse import bass_utils

    B, T, C = 4, 2048, 1024
    H, n_cores = 16, 8
    assert x.shape == (B, T, C)
    nc = _get_nc(B, T, C, H, n_cores)
    in_maps = shard_inputs(x, W_qkv, b_qkv, W_proj, b_proj, H, n_cores)
    res = bass_utils.run_bass_kernel_spmd(
        nc, in_maps, core_ids=list(range(n_cores))
    )
    return gather_output(res.results, B, T, C, n_cores)


# revision 27
# speedup vs baseline: 1.3753x; 1.0202x over previous
"""Causal self-attention on 8 Trainium2 NeuronCores (Bass/Tile).

Problem: nn_CausalSelfAttention (B=4, T=2048, C=1024, H=16 heads, fp32).

Sharding: tensor-parallel over heads for QKV projection + attention
(2 heads per core), per-batch AllGather of attention outputs (fp16,
transposed layout), then tensor-parallel over output columns for the
final projection (each core computes a 128-column slice of x@W_proj).

Schedule: software-pipelined per batch.  The attention kt-stream of
batch b is interleaved with "PE filler" groups — the QKV projection
row-tiles of batch b+1 and the output-projection row-tiles of batch
b-1 — so the tensor engine never idles while the scalar engine works
through the exp() stream (exp is the per-batch critical path; PE idle
gaps would also drop the HAM clock gate to half rate).

Layouts (feature dim on partitions everywhere):
  xT      [C, B*T]        input (fp16), replicated to all cores
  Q^T,K^T [CH, B*T]       CH = 2 heads x 64, heads stacked on
                          partitions 0:64 / 64:128
  V       [B*T, CH]       matmul lhsT for P@V, stored tiled with an
                          extra ones-column per head so the PV matmul
                          also produces softmax denominators
  S^T     [kr, 2*q] pair  scores transposed, both heads side by side in
                          one 2-bank PSUM tile; the two score matmuls
                          are K=64 row-tiled pairs (tile_position
                          (0,0)/(64,0)) that execute concurrently; one
                          exp() activation covers both heads
  attn^T  [CH, T] fp16    per-core, per-batch -> AllGather -> [C, T]
  y^T     [OC, B*T]       per-core 128-column slice of the final output

Softmax: unnormalized exp (scores are O(1)); causal mask = PE add of a
-60000 upper-triangular [128,128] constant onto the diagonal strip;
denominator from the V ones-column; division via
reciprocal_approx_fast on the PE-broadcast denominator.
All matmuls run fp16 inputs with fp32 PSUM accumulation.
"""

import numpy as np
from contextlib import ExitStack

P = 128
NQ = 512  # q/moving-operand tile width
MASKVAL = -60000.0


def build_attention_nc(B, T, C, H, n_cores):
    import concourse.bass as bass  # noqa: F401
    import concourse.bacc as bacc
    import concourse.tile as tile
    import concourse.mybir as mybir

    f32 = mybir.dt.float32
    fp16 = mybir.dt.float16
    Exp = mybir.ActivationFunctionType.Exp

    hs = C // H              # head size
    hpc = H // n_cores       # heads per core
    CH = hpc * hs            # qkv channels per core
    OC = C // n_cores        # output columns per core
    NT = B * T               # tokens
    KT_E = C // P            # contraction tiles over embedding dim
    TQ = T // NQ             # q tiles per batch
    TK = T // P              # kr tiles per batch
    TKALL = NT // P          # kr tiles over all batches
    DPB = NQ // P            # kr-tiles crossing one q-tile's diagonal
    WV = hpc * (hs + 1)      # V storage width per kr-tile (with ones cols)

    assert T % NQ == 0 and C % P == 0 and NT % NQ == 0
    assert CH == P and H % n_cores == 0 and hpc == 2 and hs == 64
    scale = 1.0 / float(np.sqrt(hs))

    nc = bacc.Bacc("TRN2", target_bir_lowering=False, debug=False,
                   num_devices=n_cores)

    xT = nc.dram_tensor("xT", [C, NT], fp16, kind="ExternalInput")
    wqkv = nc.dram_tensor("wqkv", [C, 3 * CH], fp16, kind="ExternalInput")
    bqkv = nc.dram_tensor("bqkv", [CH, 3], f32, kind="ExternalInput")
    wp = nc.dram_tensor("wp", [C, OC], fp16, kind="ExternalInput")
    bp = nc.dram_tensor("bp", [OC, 1], f32, kind="ExternalInput")
    yT = nc.dram_tensor("yT", [OC, NT], f32, kind="ExternalOutput")

    ident_np = np.eye(P, dtype=np.float16)
    # mask[p, c] = MASKVAL where kr-offset p > q-offset c (strictly lower)
    mask_np = np.where(
        np.arange(P)[:, None] > np.arange(P)[None, :],
        np.float16(MASKVAL), np.float16(0.0)).astype(np.float16)
    ident_dram = nc.inline_tensor(ident_np, name="ident_const")
    mask_dram = nc.inline_tensor(mask_np, name="mask_const")
    ones_dram = nc.inline_tensor(np.ones((P, hs), dtype=np.float16),
                                 name="ones_const")
    # Vn image with the denominator ones-columns baked in; the value
    # columns are overwritten by the V transposes at runtime
    vinit_np = np.zeros((P, TKALL * WV), dtype=np.float16)
    vinit_np.reshape(P, TKALL, hpc, hs + 1)[:, :, :, hs] = 1.0
    vinit_dram = nc.inline_tensor(vinit_np, name="vinit_const")

    with tile.TileContext(nc) as tc, ExitStack() as ctx:
        const = ctx.enter_context(tc.tile_pool(name="const", bufs=1))
        big = ctx.enter_context(tc.tile_pool(name="big", bufs=1))
        xin = ctx.enter_context(tc.tile_pool(name="xin", bufs=6))
        evac = ctx.enter_context(tc.tile_pool(name="evac", bufs=3))
        pexp = ctx.enter_context(tc.tile_pool(name="pexp", bufs=5))
        stp = ctx.enter_context(tc.tile_pool(name="stp", bufs=3, space="PSUM"))
        pvp = ctx.enter_context(tc.tile_pool(name="pvp", bufs=2, space="PSUM"))
        dram = ctx.enter_context(tc.tile_pool(name="dram", bufs=1, space="DRAM"))

        ident_t = const.tile([P, P], fp16)
        mask_sb = const.tile([P, P], fp16)
        ones_sb = const.tile([P, hs], fp16)
        bqkv_sb = const.tile([CH, 3], f32)
        bp_sb = const.tile([OC, 1], f32)
        w_sb = const.tile([P, KT_E * 3 * CH], fp16)
        wp_sb = const.tile([P, KT_E * OC], fp16)

        nc.sync.dma_start(bqkv_sb[:], bqkv[:])
        nc.sync.dma_start(ident_t[:], ident_dram[:])
        nc.sync.dma_start(mask_sb[:], mask_dram[:])
        nc.sync.dma_start(ones_sb[:], ones_dram[:])
        nc.sync.dma_start(bp_sb[:], bp[:])
        # weights k-slice by k-slice so the first QKV matmul can start
        # ~1us after the first two transfers instead of after the full load
        for k in range(KT_E):
            nc.sync.dma_start(
                w_sb[:, k * 3 * CH:(k + 1) * 3 * CH],
                wqkv[k * P:(k + 1) * P, :],
            )

        QT = big.tile([P, NT], fp16)
        KTp = big.tile([P, NT], fp16)
        VT = big.tile([CH, NT], fp16)
        Vn = big.tile([P, TKALL * WV], fp16)

        def load_late_consts():
            # not needed until the attention-PV / projection phases
            # (ones columns of V via one contiguous DMA of the full Vn
            # image; value columns get overwritten later)
            nc.sync.dma_start(Vn[:], vinit_dram[:])
            nc.sync.dma_start(
                wp_sb[:].rearrange("p (k m) -> p k m", k=KT_E),
                wp[:].rearrange("(k p) m -> p k m", p=P),
            )

        # ---- QKV projection for one 512-token row-tile; emitted either
        # directly (batch 0) or as filler groups inside an attention batch
        def qkv_row_fillers(n):
            ns = n * NQ
            xt_box = []

            def load_x():
                xt = xin.tile([P, KT_E * NQ], fp16, tag="xcol")
                nc.sync.dma_start(
                    xt[:].rearrange("p (k q) -> p k q", k=KT_E),
                    xT[:, ns:ns + NQ].rearrange("(k p) q -> p k q", p=P),
                )
                xt_box.append(xt)

            def mm_group(m):
                def go():
                    xt = xt_box[0]
                    ps = stp.tile([P, 2 * NQ], f32, tag="st")
                    for k in range(KT_E):
                        nc.tensor.matmul(
                            ps[:, 0:NQ],
                            lhsT=w_sb[:, k * 3 * CH + m * CH:
                                      k * 3 * CH + (m + 1) * CH],
                            rhs=xt[:, k * NQ:(k + 1) * NQ],
                            start=(k == 0),
                            stop=(k == KT_E - 1),
                        )
                    dst = (QT, KTp, VT)[m]
                    nc.vector.tensor_scalar_add(dst[:, ns:ns + NQ],
                                                ps[:, 0:NQ],
                                                bqkv_sb[:, m:m + 1])
                return go

            def transposes():
                tp = stp.tile([P, DPB * CH], fp16, tag="st", name="tp")
                for j in range(DPB):
                    nc.tensor.transpose(
                        tp[:, j * CH:(j + 1) * CH],
                        VT[:, (n * DPB + j) * P:(n * DPB + j + 1) * P],
                        ident_t[:],
                    )
                vi0 = n * DPB
                dst = Vn[:, vi0 * WV:(vi0 + DPB) * WV].rearrange(
                    "p (v h d) -> p v h d", h=hpc, d=hs + 1
                )[:, :, :, 0:hs]
                nc.vector.tensor_copy(dst, tp[:].rearrange(
                    "p (v h d) -> p v h d", h=hpc, d=hs))

            load_x()
            return [mm_group(0), mm_group(1), mm_group(2), transposes]

        # ---- per-batch AllGather pieces: every batch is gathered in two
        # halves so the projection rows unblock early; the last batch's
        # second half is further split per q-tile to shrink the tail
        pieces = []  # per batch: list of (start_qt, n_qt)
        for b in range(B):
            if b == B - 1:
                pieces.append([(0, 2), (2, 1), (3, 1)])
            else:
                pieces.append([(0, 2), (2, 2)])
        cc_ins, cc_outs = [], []
        for b in range(B):
            cc_ins.append([dram.tile([CH, n * NQ], fp16,
                                     name=f"ccin{b}_{q0}")
                           for (q0, n) in pieces[b]])
            cc_outs.append([dram.tile([n_cores * CH, n * NQ], fp16,
                                      addr_space="Shared",
                                      name=f"ccout{b}_{q0}")
                            for (q0, n) in pieces[b]])

        def piece_of(b, qt):
            for i, (q0, n) in enumerate(pieces[b]):
                if q0 <= qt < q0 + n:
                    return i, (qt - q0) * NQ
            raise AssertionError

        def issue_ag(b, i):
            nc.gpsimd.collective_compute(
                "AllGather",
                mybir.AluOpType.bypass,
                replica_groups=[list(range(n_cores))],
                ins=[cc_ins[b][i][:].opt()],
                outs=[cc_outs[b][i][:].opt()],
            )

        def proj_row_fillers(b):
            # output projection of batch b, one filler per 512-token row
            def row(n):
                def go():
                    i, off = piece_of(b, n)
                    rt = xin.tile([P, KT_E * NQ], fp16, tag="xcol")
                    # issue on the ACT HWDGE queue so a wait on the
                    # AllGather doesn't head-of-line block the sync queue
                    nc.scalar.dma_start(
                        rt[:].rearrange("p (k q) -> p k q", k=KT_E),
                        cc_outs[b][i][:, off:off + NQ]
                        .rearrange("(k p) q -> p k q", p=P),
                    )
                    ps = stp.tile([P, 2 * NQ], f32, tag="st")
                    for k in range(KT_E):
                        nc.tensor.matmul(
                            ps[0:OC, 0:NQ],
                            lhsT=wp_sb[:, k * OC:(k + 1) * OC],
                            rhs=rt[:, k * NQ:(k + 1) * NQ],
                            start=(k == 0),
                            stop=(k == KT_E - 1),
                        )
                    yo = evac.tile([OC, NQ], f32, tag="yo")
                    nc.vector.tensor_scalar_add(yo[:], ps[0:OC, 0:NQ],
                                                bp_sb[:, 0:1])
                    nc.sync.dma_start(yT[:, b * T + n * NQ:
                                         b * T + n * NQ + NQ], yo[:])
                return go

            return [row(n) for n in range(T // NQ)]

        def attention(b, fillers):
            """kt-stream of batch b with PE filler groups interleaved."""
            fillers = list(fillers)
            # pace the fillers across the whole batch: later q-tiles have
            # the longest exp() stretches and need PE work the most
            ndrains = [sum(1 for kt in range(DPB * qt + DPB) if kt % 2 == 1)
                       + 1 for qt in range(TQ)]
            budget = [0]
            budget[0] = sum(ndrains)

            def drain(force=False):
                if not force:
                    budget[0] -= 1
                if fillers and (force or len(fillers) >= budget[0]):
                    f = fillers.pop(0)
                    if f is not None:
                        f()

            for qt in range(TQ):
                qs = b * T + qt * NQ
                nkt = DPB * qt + DPB
                pvs = [pvp.tile([P, NQ], f32, tag="pv", name=f"pv{_h}")
                       for _h in range(hpc)]
                pes = {}

                def issue_st(kt, qt=qt, qs=qs, pes=pes):
                    ks = b * T + kt * P
                    diag = kt >= DPB * qt
                    j = kt - DPB * qt
                    c0 = j * P if diag else 0
                    st = stp.tile([P, 2 * NQ], f32, tag="st")
                    for hh in range(hpc):
                        nc.tensor.matmul(
                            st[:, hh * NQ + c0:(hh + 1) * NQ],
                            lhsT=KTp[hh * hs:(hh + 1) * hs, ks:ks + P],
                            rhs=QT[hh * hs:(hh + 1) * hs, qs + c0:qs + NQ],
                            start=True,
                            stop=not diag,
                            tile_position=(hh * hs, 0),
                        )
                    if diag:
                        for hh in range(hpc):
                            nc.tensor.matmul(
                                st[:, hh * NQ + c0:hh * NQ + c0 + P],
                                lhsT=ident_t[:],
                                rhs=mask_sb[:],
                                start=False,
                                stop=True,
                            )
                    pe_t = pexp.tile([P, 2 * NQ], fp16, tag="pe")
                    nc.scalar.activation(
                        pe_t[:].rearrange("p (h q) -> p h q", h=hpc)
                        [:, :, c0:NQ],
                        st[:].rearrange("p (h q) -> p h q", h=hpc)
                        [:, :, c0:NQ],
                        Exp, scale=scale)
                    pes[kt] = (pe_t, c0)

                def issue_pv(kt, nkt=nkt, pvs=pvs, pes=pes):
                    vi = b * TK + kt
                    pe_t, c0 = pes.pop(kt)
                    for hh in range(hpc):
                        nc.tensor.matmul(
                            pvs[hh][0:hs + 1, c0:NQ],
                            lhsT=Vn[:, vi * WV + hh * (hs + 1):
                                    vi * WV + (hh + 1) * (hs + 1)],
                            rhs=pe_t[:, hh * NQ + c0:(hh + 1) * NQ],
                            start=(kt == 0),
                            stop=(kt == nkt - 1),
                        )

                for kt in range(nkt):
                    issue_st(kt)
                    if kt % 2 == 1:
                        drain()
                    if kt >= 2:
                        issue_pv(kt - 2)
                for kt in range(max(0, nkt - 2), nkt):
                    issue_pv(kt)

                # normalization: denominators -> broadcast -> reciprocal ->
                # scale; a filler between the DVE dens copy and the PE
                # broadcast hides the DVE latency
                dens = evac.tile([1, 2 * NQ], fp16, tag="den", bufs=3)
                for hh in range(hpc):
                    nc.vector.tensor_copy(dens[:, hh * NQ:(hh + 1) * NQ],
                                          pvs[hh][hs:hs + 1, :])
                drain()
                bc = stp.tile([P, 2 * NQ], f32, tag="st")
                for hh in range(hpc):
                    nc.tensor.matmul(
                        bc[0:hs, hh * NQ:(hh + 1) * NQ],
                        lhsT=ones_sb[0:1, :],
                        rhs=dens[:, hh * NQ:(hh + 1) * NQ],
                        start=True,
                        stop=True,
                    )
                recs = evac.tile([hs, 2 * NQ], f32, tag="rec", bufs=2)
                nc.vector.reciprocal_approx_fast(recs[:], bc[0:hs, :])
                pi, off = piece_of(b, qt)
                for hh in range(hpc):
                    ao = evac.tile([hs, NQ], fp16, tag="ao")
                    nc.vector.tensor_mul(ao[:], pvs[hh][0:hs, :],
                                         recs[:, hh * NQ:(hh + 1) * NQ])
                    nc.sync.dma_start(
                        cc_ins[b][pi][hh * hs:(hh + 1) * hs, off:off + NQ],
                        ao[:],
                    )
                if qt == pieces[b][pi][0] + pieces[b][pi][1] - 1:
                    issue_ag(b, pi)
            # leftover fillers (also give the last AG time to land before
            # the dependent projection rows run)
            while fillers:
                drain()

        # ---- main schedule ----
        row0 = qkv_row_fillers(0)
        row0[0]()
        load_late_consts()
        for g in row0[1:]:
            g()
        for g in qkv_row_fillers(1) + qkv_row_fillers(2) + qkv_row_fillers(3):
            g()
        proj_last = proj_row_fillers(B - 1)
        for b in range(B):
            if b + 1 < B:
                qkv = []
                for n in range(TQ):
                    qkv += qkv_row_fillers((b + 1) * TQ + n)
            else:
                qkv = [None] * 16
            # proj rows of b-1: rows 0,1 depend on the first AG piece of
            # b-1 (issued mid-attention(b-1), long done); rows 2,3 on the
            # second piece (issued at attention(b-1) end) -> place late
            proj = proj_row_fillers(b - 1) if b >= 1 else [None] * 4
            fillers = qkv[0:8] + proj[0:2] + qkv[8:16] + proj[2:4]
            if b + 1 == B:
                # last batch: slot its own first-piece projection rows at
                # the very end of the stream (their AG is issued at qt1)
                fillers += proj_last[0:2]
            attention(b, fillers)
        # tail: the remaining projection rows of the last batch, each
        # gated by its own per-q-tile AllGather piece
        for g in proj_last[2:4]:
            g()

    nc.compile()
    return nc


def shard_inputs(x, W_qkv, b_qkv, W_proj, b_proj, H, n_cores):
    B, T, C = x.shape
    hs = C // H
    hpc = H // n_cores
    CH = hpc * hs
    OC = C // n_cores
    x2 = np.asarray(x, dtype=np.float32).reshape(B * T, C)
    xT = np.ascontiguousarray(x2.T.astype(np.float16))
    W_qkv = np.asarray(W_qkv, dtype=np.float32)
    b_qkv = np.asarray(b_qkv, dtype=np.float32)
    W_proj = np.asarray(W_proj, dtype=np.float32)
    b_proj = np.asarray(b_proj, dtype=np.float32)
    in_maps = []
    for i in range(n_cores):
        sl = slice(i * CH, (i + 1) * CH)
        wqkv_i = np.ascontiguousarray(np.concatenate(
            [W_qkv[:, sl], W_qkv[:, C:][:, sl], W_qkv[:, 2 * C:][:, sl]],
            axis=1).astype(np.float16))
        bqkv_i = np.ascontiguousarray(np.stack(
            [b_qkv[sl], b_qkv[C:][sl], b_qkv[2 * C:][sl]], axis=1))
        wp_i = np.ascontiguousarray(
            W_proj[:, i * OC:(i + 1) * OC].astype(np.float16))
        bp_i = np.ascontiguousarray(b_proj[i * OC:(i + 1) * OC].reshape(OC, 1))
        in_maps.append({"xT": xT, "wqkv": wqkv_i, "bqkv": bqkv_i,
                        "wp": wp_i, "bp": bp_i})
    return in_maps


def gather_output(results, B, T, C, n_cores):
    yT = np.concatenate([results[i]["yT"] for i in range(n_cores)], axis=0)
    return np.ascontiguousarray(yT.T).reshape(B, T, C).astype(np.float32)


_NC_CACHE = {}


def _get_nc(B, T, C, H, n_cores):
    key = (B, T, C, H, n_cores)
    if key not in _NC_CACHE:
        _NC_CACHE[key] = build_attention_nc(B, T, C, H, n_cores)
    return _NC_CACHE[key]


def kernel(x, W_qkv, b_qkv, W_proj, b_proj):
    from concourse import bass_utils

    B, T, C = 4, 2048, 1024
    H, n_cores = 16, 8
    assert x.shape == (B, T, C)
    nc = _get_nc(B, T, C, H, n_cores)
    in_maps = shard_inputs(x, W_qkv, b_qkv, W_proj, b_proj, H, n_cores)
    res = bass_utils.run_bass_kernel_spmd(
        nc, in_maps, core_ids=list(range(n_cores))
    )
    return gather_output(res.results, B, T, C, n_cores)
